# revision 11
# baseline (speedup 1.0000x reference)
"""Trainium2 Bass kernel v3 for the GNN edge-MLP (nn_BMA_update):

    out[e] = relu( relu([x[src]|x[dst]] @ W_nnn + b_nnn)
                 + relu(edge_attr @ W_root + b_root) ) @ W_out -> relu

Arch (edge-parallel across 8 NeuronCores, fp16 data path):
  - Host shards edges into 8 contiguous slices; within each slice, edges are
    bucketed by (src_bank, dst_bank) over 4 banks of 25000 node rows (int16
    gather indices), padded to a fixed per-bucket cap.
  - x is shipped once as fp16 [100000, 128] (cols 0-63 = x, rest zero);
    non-transpose dma_gather (the HW-validated mode) lands rows edge-major;
    per 512-edge tile, 8 PE transposes move src rows into partitions 0-63 and
    dst rows into 64-127 of one PSUM tile (fully unrolled loops make all AP
    offsets static, so no staging copies are needed), then one DVE copy emits
    the fp16 pairT tile.
  - h1 = W1^T pairT; h2 = Wroot^T attrT (attr is host-pre-transposed fp16,
    loaded [64, 2048] per 4-tile group); relu+bias fused: hs1 on ACT, hs2 on
    DVE (tensor_scalar add+max); o = W3^T hs1 + W3^T hs2 (PSUM accumulate);
    final relu+bias on ACT emits fp16 [128, 2048] groups stored feature-major
    with 2KB+ DMA descriptors; host de-transposes and un-permutes.
"""
import numpy as np

import concourse.bacc as bacc
import concourse.mybir as mybir
import concourse.tile as tile
from concourse import bass_utils
from concourse.masks import make_identity

N_NODES = 100000
N_EDGES = 1600000
NODE_C = 64
EDGE_C = 64
HIDDEN_C = 128
OUT_C = 128
N_CORES = 8
N_BANKS = 4
BANK = N_NODES // N_BANKS          # 25000 (< 32768 so int16 indices work)
N_BUCKETS = N_BANKS * N_BANKS      # 16
MEGA = 512                         # edges per matmul tile (PSUM bank width)
GROUP = 1024                       # edges per attr/out DMA group (2 tiles)
DEFAULT_CAP = 13312                # divisible by 1024; mean bucket load 12500
F32, F16, I16 = mybir.dt.float32, mybir.dt.float16, mybir.dt.int16

_BUILD_CACHE = {}


def _build_kernel(cap, n_queues=4, repeat=1, split=2):
    key = (cap, n_queues, repeat, split)
    if key in _BUILD_CACHE:
        return _BUILD_CACHE[key]
    assert cap % GROUP == 0
    n_groups = cap // GROUP
    S = cap // 16
    assert (cap // split) % 128 == 0 and S % split == 0

    nc = bacc.Bacc("TRN2", target_bir_lowering=False, debug=False,
                   num_swdge_queues=n_queues)
    xp = nc.dram_tensor("xp", (N_NODES, 128), F16, kind="ExternalInput")
    sidx = nc.dram_tensor("sidx", (N_BUCKETS, 128, S), I16, kind="ExternalInput")
    didx = nc.dram_tensor("didx", (N_BUCKETS, 128, S), I16, kind="ExternalInput")
    attr = nc.dram_tensor("attr", (N_BUCKETS, n_groups, EDGE_C, GROUP), F16,
                          kind="ExternalInput")
    Wnnn = nc.dram_tensor("Wnnn", (2 * NODE_C, HIDDEN_C), F16, kind="ExternalInput")
    Wroot = nc.dram_tensor("Wroot", (EDGE_C, HIDDEN_C), F16, kind="ExternalInput")
    Wout = nc.dram_tensor("Wout", (HIDDEN_C, OUT_C), F16, kind="ExternalInput")
    bnnn = nc.dram_tensor("bnnn", (HIDDEN_C, 1), F32, kind="ExternalInput")
    broot = nc.dram_tensor("broot", (HIDDEN_C, 1), F32, kind="ExternalInput")
    bout = nc.dram_tensor("bout", (OUT_C, 1), F32, kind="ExternalInput")
    out = nc.dram_tensor("out", (N_BUCKETS, n_groups, OUT_C, GROUP), F16,
                         kind="ExternalOutput")

    with tile.TileContext(nc) as tc:
        with (
            tc.tile_pool(name="const", bufs=1) as cpool,
            tc.tile_pool(name="gat", bufs=2) as gpool,
            tc.tile_pool(name="attrp", bufs=3) as apool,
            tc.tile_pool(name="work", bufs=3) as wpool,
            tc.tile_pool(name="outp", bufs=2) as opool,
            tc.tile_pool(name="tp_ps", bufs=2, space="PSUM") as tp_ps,
            tc.tile_pool(name="h1_ps", bufs=2, space="PSUM") as h1_ps,
            tc.tile_pool(name="h2_ps", bufs=2, space="PSUM") as h2_ps,
            tc.tile_pool(name="o_ps", bufs=2, space="PSUM") as o_ps,
        ):
            identf = cpool.tile([128, 128], F32)
            make_identity(nc, identf[:])
            ident = cpool.tile([128, 128], F16)
            nc.vector.tensor_copy(ident[:], identf[:])
            w1 = cpool.tile([128, HIDDEN_C], F16)
            nc.sync.dma_start(out=w1[:], in_=Wnnn.ap())
            w2 = cpool.tile([EDGE_C, HIDDEN_C], F16)
            nc.sync.dma_start(out=w2[:], in_=Wroot.ap())
            w3 = cpool.tile([128, OUT_C], F16)
            nc.sync.dma_start(out=w3[:], in_=Wout.ap())
            b1 = cpool.tile([HIDDEN_C, 1], F32)
            nc.sync.dma_start(out=b1[:], in_=bnnn.ap())
            b2 = cpool.tile([HIDDEN_C, 1], F32)
            nc.sync.dma_start(out=b2[:], in_=broot.ap())
            b3 = cpool.tile([OUT_C, 1], F32)
            nc.sync.dma_start(out=b3[:], in_=bout.ap())

            def issue_gathers(b):
                # split each src/dst gather into `split` column-range halves on
                # distinct SWDGE queues: 2*split gathers run concurrently per
                # bucket (measured ~3.6ns/desc at 4 queues vs ~5.4 at 2).
                sb, db = b // N_BANKS, b % N_BANKS
                sidx_sb = gpool.tile([128, S], I16, tag="sidx")
                nc.sync.dma_start(out=sidx_sb[:], in_=sidx.ap()[b])
                didx_sb = gpool.tile([128, S], I16, tag="didx")
                nc.sync.dma_start(out=didx_sb[:], in_=didx.ap()[b])
                sgat = gpool.tile([128, (cap // 128) * 128], F16, tag="sgat")
                dgat = gpool.tile([128, (cap // 128) * 128], F16, tag="dgat")
                cs, ss = cap // split, S // split
                for h in range(split):
                    nc.gpsimd.dma_gather(
                        out_ap=sgat[:, h * cs:(h + 1) * cs]
                               .rearrange("p (t f) -> p t f", f=128),
                        in_ap=xp.ap()[sb * BANK:(sb + 1) * BANK, :],
                        idxs_ap=sidx_sb[:, h * ss:(h + 1) * ss],
                        num_idxs=cs, num_idxs_reg=cs, elem_size=128,
                        single_packet=False, queue_num=(2 * h) % n_queues,
                    )
                    nc.gpsimd.dma_gather(
                        out_ap=dgat[:, h * cs:(h + 1) * cs]
                               .rearrange("p (t f) -> p t f", f=128),
                        in_ap=xp.ap()[db * BANK:(db + 1) * BANK, :],
                        idxs_ap=didx_sb[:, h * ss:(h + 1) * ss],
                        num_idxs=cs, num_idxs_reg=cs, elem_size=128,
                        single_packet=False, queue_num=(2 * h + 1) % n_queues,
                    )
                return (sgat[:].rearrange("p (t f) -> p t f", f=128),
                        dgat[:].rearrange("p (t f) -> p t f", f=128))

            from contextlib import nullcontext
            rep_cm = tc.For_i(0, repeat) if repeat > 1 else nullcontext()
            with rep_cm:
              gat_next = issue_gathers(0)
              for b in range(N_BUCKETS):
                sgat3, dgat3 = gat_next
                if b + 1 < N_BUCKETS:
                    gat_next = issue_gathers(b + 1)
                for g in range(n_groups):
                    attr_sb = apool.tile([EDGE_C, GROUP], F16, tag="attr")
                    nc.sync.dma_start(out=attr_sb[:], in_=attr.ap()[b, g])
                    out_sb = opool.tile([128, GROUP], F16, tag="out")
                    for t in range(GROUP // MEGA):
                        c0 = (g * GROUP + t * MEGA) // 128   # first 128-chunk
                        tp = tp_ps.tile([128, MEGA], F16, tag="tp", space="PSUM")
                        for j in range(4):
                            nc.tensor.transpose(
                                out=tp[0:64, j * 128:(j + 1) * 128],
                                in_=sgat3[:, c0 + j, 0:64],
                                identity=ident[:])
                            nc.tensor.transpose(
                                out=tp[64:128, j * 128:(j + 1) * 128],
                                in_=dgat3[:, c0 + j, 0:64],
                                identity=ident[:])
                        pairT = wpool.tile([128, MEGA], F16, tag="pairT")
                        nc.vector.tensor_copy(pairT[:], tp[:])
                        h1 = h1_ps.tile([128, MEGA], F32, tag="h1", space="PSUM")
                        nc.tensor.matmul(out=h1[:], lhsT=w1[:], rhs=pairT[:],
                                         start=True, stop=True)
                        h2 = h2_ps.tile([128, MEGA], F32, tag="h2", space="PSUM")
                        nc.tensor.matmul(out=h2[:], lhsT=w2[:],
                                         rhs=attr_sb[:, t * MEGA:(t + 1) * MEGA],
                                         start=True, stop=True)
                        hs1 = wpool.tile([128, MEGA], F16, tag="hs1")
                        nc.scalar.activation(hs1[:], h1[:],
                                             mybir.ActivationFunctionType.Relu,
                                             bias=b1[:])
                        hs2 = wpool.tile([128, MEGA], F16, tag="hs2")
                        nc.vector.tensor_scalar(
                            out=hs2[:], in0=h2[:], scalar1=b2[:], scalar2=0.0,
                            op0=mybir.AluOpType.add, op1=mybir.AluOpType.max)
                        o = o_ps.tile([128, MEGA], F32, tag="o", space="PSUM")
                        nc.tensor.matmul(out=o[:], lhsT=w3[:], rhs=hs1[:],
                                         start=True, stop=False)
                        nc.tensor.matmul(out=o[:], lhsT=w3[:], rhs=hs2[:],
                                         start=False, stop=True)
                        nc.scalar.activation(out_sb[:, t * MEGA:(t + 1) * MEGA],
                                             o[:],
                                             mybir.ActivationFunctionType.Relu,
                                             bias=b3[:])
                    nc.sync.dma_start(out=out.ap()[b, g], in_=out_sb[:])
    nc.compile()
    _BUILD_CACHE[key] = nc
    return nc


def _host_prep(src_all, dst_all, edge_attr, cap, n_cores=N_CORES, split=2):
    E = src_all.shape[0]
    Ec = E // n_cores
    n_groups = cap // GROUP
    cs = cap // split
    per_core = []
    for c in range(n_cores):
        lo, hi = c * Ec, (c + 1) * Ec
        src, dst = src_all[lo:hi], dst_all[lo:hi]
        bucket = (src // BANK) * N_BANKS + (dst // BANK)
        order = np.argsort(bucket, kind="stable")
        counts = np.bincount(bucket, minlength=N_BUCKETS)
        if counts.max() > cap:
            return None, int(counts.max())
        sorted_bucket = bucket[order]
        within = np.arange(Ec) - np.concatenate(([0], np.cumsum(counts)))[sorted_bucket]
        # balance real edges across the `split` gather halves (k%split picks
        # the half, k//split the slot) so each half's trailing -1 pad is equal
        # and the SWDGE queues carry even descriptor loads.
        within = (within % split) * cs + within // split
        pos = sorted_bucket * cap + within
        sloc = np.zeros(N_BUCKETS * cap, np.int16)
        dloc = np.zeros(N_BUCKETS * cap, np.int16)
        sloc[pos] = (src[order] % BANK).astype(np.int16)
        dloc[pos] = (dst[order] % BANK).astype(np.int16)
        S = cap // 16

        def wrap(a):
            w = a.reshape(N_BUCKETS, S, 16).transpose(0, 2, 1)
            return np.ascontiguousarray(np.tile(w, (1, 8, 1)))

        attr_p = np.zeros((N_BUCKETS * cap, EDGE_C), np.float16)
        attr_p[pos] = edge_attr[lo:hi][order]
        attr_t = np.ascontiguousarray(
            attr_p.reshape(N_BUCKETS, n_groups, GROUP, EDGE_C)
                  .transpose(0, 1, 3, 2))
        per_core.append(dict(sidx=wrap(sloc), didx=wrap(dloc), attr=attr_t,
                             meta=(order, pos)))
    return per_core, None


def kernel(x, edge_index, edge_attr, W_nnn, b_nnn, W_root, b_root, W_out, b_out,
           _repeat=1, _n_runs=1, _n_queues=4, _split=2):
    x = np.asarray(x, np.float32)
    edge_index = np.asarray(edge_index)
    edge_attr = np.asarray(edge_attr, np.float16)
    W_nnn = np.asarray(W_nnn, np.float16)
    W_root = np.asarray(W_root, np.float16)
    W_out = np.asarray(W_out, np.float16)
    b_nnn = np.asarray(b_nnn, np.float32).reshape(-1, 1)
    b_root = np.asarray(b_root, np.float32).reshape(-1, 1)
    b_out = np.asarray(b_out, np.float32).reshape(-1, 1)
    E = edge_index.shape[1]
    src_all = np.ascontiguousarray(edge_index[0]).astype(np.int64)
    dst_all = np.ascontiguousarray(edge_index[1]).astype(np.int64)

    xp = np.zeros((N_NODES, 128), np.float16)
    xp[:, 0:64] = x.astype(np.float16)

    cap = DEFAULT_CAP
    while True:
        per_core, max_count = _host_prep(src_all, dst_all, edge_attr, cap,
                                         split=_split)
        if per_core is not None:
            break
        cap = ((max_count + GROUP - 1) // GROUP) * GROUP  # rare: grow and retry

    nc = _build_kernel(cap, n_queues=_n_queues, repeat=_repeat, split=_split)
    common = {"xp": xp, "Wnnn": W_nnn, "Wroot": W_root, "Wout": W_out,
              "bnnn": b_nnn, "broot": b_root, "bout": b_out}
    in_maps = [{**common, "sidx": p["sidx"], "didx": p["didx"], "attr": p["attr"]}
               for p in per_core]
    res = None
    times = []
    for _ in range(max(1, _n_runs)):
        import time as _time
        t0 = _time.perf_counter()
        res = bass_utils.run_bass_kernel_spmd(nc, in_maps,
                                              core_ids=list(range(N_CORES)))
        times.append(_time.perf_counter() - t0)
    kernel.last_wall_times = times

    Ec = E // N_CORES
    n_groups = cap // GROUP
    full = np.empty((E, OUT_C), np.float32)
    for c in range(N_CORES):
        order, pos = per_core[c]["meta"]
        o = res.results[c]["out"].reshape(N_BUCKETS, n_groups, OUT_C, GROUP)
        o = o.transpose(0, 1, 3, 2).reshape(N_BUCKETS * cap, OUT_C)
        full[c * Ec + order] = o[pos].astype(np.float32)
    return full
